# revision 30
# baseline (speedup 1.0000x reference)
"""Multi-head self-attention (B=4, T=2048, D=1024, H=16) on 8 Trainium2
NeuronCores, head-parallel (2 heads per core).

Per-core dataflow (all bf16 matmuls, fp32 PSUM accumulation):
  xT[b] (host-pretransposed [D, T] bf16) -> SBUF
  qT/kT = w_{q,k}^T @ x^T          [128=2*dk, T]   (transposed layout)
  v     = x @ w_v                  [T, 128=2*dk]   (natural layout, +ones col)
  S^T   = kT.T @ qT per (k-block, q-panel), two heads row-tiled on the PE
  P^T   = exp(S^T / 8) on ACT (no max subtraction; scores are O(6))
  causal: strictly-upper k-blocks skipped; diagonal superblock masked by a
  0/1 tril multiply (gpsimd)
  PV    = v_aug.T @ P^T -> [65, W] PSUM (row 64 = softmax denominator l)
  attn_T = PV * recip(l) (broadcast), head1 shifted to partitions 64-127
  out_partial = attn_T.T @ w_proj_rows -> HBM fp32
Host: verifies the mask is causal, pre-transposes/casts x, sums the 8
partial outputs.
"""
import numpy as np
import ml_dtypes

B, T, D, H, DK = 4, 2048, 1024, 16, 64
NCORES = 8
CD = 128          # per-core head dims (2 heads x 64)
W = 512           # q panel width
NCH = D // 128    # contraction chunks for qkv
VS = 66           # v_aug per-head stride: 64 v cols + 1 ones + 1 pad

bf16 = ml_dtypes.bfloat16
_PROG_CACHE = {}
LAST_RESULT = None


def _install_ntff_hook():
    """Register antenv.axon_hooks (NTFF profiling) if the image lacks it."""
    import contextlib
    import ctypes
    import sys
    import types

    try:
        from antenv.axon_hooks import get_axon_ntff_profile_hook  # noqa: F401
        return
    except ImportError:
        pass

    lib = ctypes.CDLL("/opt/axon/libaxon_pjrt.so")
    if not hasattr(lib, "axon_start_nrt_profile"):
        return
    lib.axon_start_nrt_profile.argtypes = [ctypes.POINTER(ctypes.c_int64), ctypes.c_size_t]
    lib.axon_start_nrt_profile.restype = ctypes.c_int64
    lib.axon_stop_nrt_profile.argtypes = [ctypes.c_char_p]
    lib.axon_stop_nrt_profile.restype = ctypes.c_int64

    @contextlib.contextmanager
    def hook(output_dir, device_ids=None):
        import jax

        jax.devices()
        if device_ids:
            ids = (ctypes.c_int64 * len(device_ids))(*device_ids)
            rc = lib.axon_start_nrt_profile(ids, len(device_ids))
        else:
            rc = lib.axon_start_nrt_profile(None, 0)
        if rc != 0:
            raise RuntimeError(f"axon_start_nrt_profile rc={rc}")
        try:
            yield
        finally:
            n = lib.axon_stop_nrt_profile(str(output_dir).encode())
            print(f"profile: {n} file(s) written to {output_dir}", file=sys.stderr)

    mod = types.ModuleType("antenv.axon_hooks")
    mod.get_axon_ntff_profile_hook = lambda: hook
    mod.set_axon_ntff_profile_hook = lambda h: None
    sys.modules["antenv.axon_hooks"] = mod
    import antenv

    antenv.axon_hooks = mod


def build_program(Bv=B, Tv=T):
    import concourse.mybir as mybir
    import concourse.tile as tile
    from concourse import bacc, library_config

    dt = mybir.dt
    f32, b16 = dt.float32, dt.bfloat16
    NPANEL = Tv // W
    NTOK = Tv // 128
    NKB = Tv // 128

    nc = bacc.Bacc()
    xt_d = nc.declare_dram_parameter("xt", [Bv, D, Tv], b16, isOutput=False)
    wq_d = nc.declare_dram_parameter("wq", [D, CD], b16, isOutput=False)
    wk_d = nc.declare_dram_parameter("wk", [D, CD], b16, isOutput=False)
    wv_d = nc.declare_dram_parameter("wv", [D, CD], b16, isOutput=False)
    wp_d = nc.declare_dram_parameter("wp", [CD, D], b16, isOutput=False)
    mk_d = nc.declare_dram_parameter("maskt", [W, 2 * W], b16, isOutput=False)
    out_d = nc.declare_dram_parameter("out", [Bv, Tv, D], f32, isOutput=True)

    Exp = mybir.ActivationFunctionType.Exp

    with tile.TileContext(nc) as tc:
        with (
            tc.tile_pool(name="const", bufs=1) as constp,
            tc.tile_pool(name="xt", bufs=2) as xtp,
            tc.tile_pool(name="qk", bufs=2) as qkp,
            tc.tile_pool(name="vv", bufs=2) as vvp,
            tc.tile_pool(name="at", bufs=2) as atp,
            tc.tile_pool(name="raw", bufs=2 * NPANEL + 2) as rawp,
            tc.tile_pool(name="pt", bufs=6) as ptp,
            tc.tile_pool(name="ell", bufs=2) as ellp,
            tc.tile_pool(name="rl", bufs=10) as rlp,
            tc.tile_pool(name="bc", bufs=4) as bcp,
            tc.tile_pool(name="stg", bufs=3) as stgp,
            tc.tile_pool(name="osb", bufs=3) as osbp,
            tc.tile_pool(name="mm", bufs=2, space="PSUM") as mmp,
            tc.tile_pool(name="qs", bufs=2, space="PSUM") as qsp,
            tc.tile_pool(name="pv", bufs=2, space="PSUM") as pvp,
        ):
            # gpsimd ucode library with TensorTensor + PartitionBroadcast
            nc.gpsimd.load_library(library_config.proxy)

            # --- constants: weights + causal mask tile ---
            wq_sb = constp.tile([128, NCH * CD], b16, tag="wq")
            wk_sb = constp.tile([128, NCH * CD], b16, tag="wk")
            wv_sb = constp.tile([128, NCH * CD], b16, tag="wv")
            for w_d, w_sb in ((wq_d, wq_sb), (wk_d, wk_sb), (wv_d, wv_sb)):
                nc.vector.dma_start(
                    w_sb[:].rearrange("p (c m) -> p c m", c=NCH),
                    w_d[:].rearrange("(c p) m -> p c m", p=128))
            wp_sb = constp.tile([128, D], b16, tag="wp")
            nc.vector.dma_start(wp_sb[:], wp_d[:])
            # mask chunk j: [128, 2W] = within-block tril pattern, duplicated
            # for the two heads packed side by side in the score tile
            mask_sb = constp.tile([128, 4 * 2 * W], b16, tag="mask")
            nc.vector.dma_start(
                mask_sb[:].rearrange("p (j m) -> p j m", j=4),
                mk_d[:].rearrange("(j p) m -> p j m", p=128))

            state = {}

            def emit_qkv(b):
                # load x^T for this batch, then qT/kT [2*dk, Tv] and v_aug
                xt_sb = xtp.tile([128, NCH * Tv], b16, tag="xt")
                HT = Tv // 2
                for ch in range(NCH):
                    for hf in range(2):
                        nc.sync.dma_start(
                            xt_sb[:, ch * Tv + hf * HT: ch * Tv + (hf + 1) * HT],
                            xt_d[b, ch * 128:(ch + 1) * 128, hf * HT:(hf + 1) * HT])
                qT = qkp.tile([128, Tv], b16, tag="qT")
                kT = qkp.tile([128, Tv], b16, tag="kT")
                for w_sb, dst in ((wq_sb, qT), (wk_sb, kT)):
                    for p in range(Tv // W):
                        ps = mmp.tile([128, W], f32, tag="mm")
                        for ch in range(NCH):
                            nc.tensor.matmul(
                                ps[:], w_sb[:, ch * CD:(ch + 1) * CD],
                                xt_sb[:, ch * Tv + p * W: ch * Tv + (p + 1) * W],
                                start=(ch == 0), stop=(ch == NCH - 1))
                        nc.vector.tensor_copy(dst[:, p * W:(p + 1) * W], ps[:])
                v_sb = vvp.tile([128, NTOK * 2 * VS], b16, tag="v")
                vr = v_sb[:].rearrange("p (n h s) -> p n h s", h=2, s=VS)
                nc.vector.memset(vr[:, :, :, 64:65], 1.0)
                for kb0 in range(0, NTOK, 4):
                    ps = mmp.tile([128, 4 * CD], f32, tag="mm", name="vps")
                    for kb in range(kb0, kb0 + 4):
                        for ch in range(NCH):
                            nc.tensor.matmul(
                                ps[:, (kb - kb0) * CD:(kb - kb0 + 1) * CD],
                                xt_sb[:, ch * Tv + kb * 128: ch * Tv + kb * 128 + 128],
                                wv_sb[:, ch * CD:(ch + 1) * CD],
                                start=(ch == 0), stop=(ch == NCH - 1))
                    nc.vector.tensor_copy(
                        vr[:, kb0:kb0 + 4, :, 0:64],
                        ps[:].rearrange("p (n h s) -> p n h s", h=2, s=64))
                state[b] = {"qT": qT, "kT": kT, "vr": vr}

            def emit_attention(b):
                # PE stream software-pipelined: QK(kb+1) before PV(kb)
                st = state[b]
                qT, kT, vr = st["qT"], st["kT"], st["vr"]
                lmat = ellp.tile([2 * NPANEL, W], f32, tag="lmat")
                raws = {}
                for p in range(NPANEL):
                    pv_ps = [pvp.tile([65, W], f32, tag="pv", name=f"pv{h}") for h in range(2)]
                    nkb = 4 * (p + 1)
                    pts = {}

                    def emit_pv(kb, nkb=nkb, pv_ps=pv_ps, pts=pts):
                        pt = pts.pop(kb)
                        for h in range(2):
                            nc.tensor.matmul(
                                pv_ps[h][0:65, :], vr[:, kb, h, 0:65],
                                pt[:, h * W:(h + 1) * W],
                                start=(kb == 0), stop=(kb == nkb - 1))

                    for kb in range(nkb):
                        qk = qsp.tile([128, 2 * W], f32, tag="qs", name="qk")
                        for h in range(2):
                            nc.tensor.matmul(
                                qk[:, h * W:(h + 1) * W],
                                kT[64 * h:64 * (h + 1), kb * 128:(kb + 1) * 128],
                                qT[64 * h:64 * (h + 1), p * W:(p + 1) * W],
                                start=True, stop=True, tile_position=(64 * h, 0))
                        if kb > 1:
                            emit_pv(kb - 2)
                        pt = ptp.tile([128, 2 * W], b16, tag="pt")
                        nc.scalar.activation(pt[:], qk[:], Exp, scale=0.125)
                        if kb >= 4 * p:
                            j = kb - 4 * p
                            nc.vector.tensor_mul(pt[:], pt[:], mask_sb[:, j * 2 * W:(j + 1) * 2 * W])
                        pts[kb] = pt
                    emit_pv(nkb - 2)
                    emit_pv(nkb - 1)
                    for h in range(2):
                        raw = rawp.tile([65, W], f32, tag="raw")
                        nc.vector.tensor_copy(raw[:], pv_ps[h][0:65, :])
                        nc.gpsimd.dma_start(lmat[h * NPANEL + p: h * NPANEL + p + 1, :], raw[64:65, :])
                        raws[(h, p)] = raw
                st["lmat"] = lmat
                st["raws"] = raws

            def emit_scale(b):
                # recip(l), broadcast, scale attn_T (head1 shifted via DMA)
                st = state[b]
                recipl = ellp.tile([2 * NPANEL, W], f32, tag="recipl")
                nc.vector.reciprocal(recipl[:], st["lmat"][:])
                attnT = atp.tile([128, Tv], b16, tag="attnT")
                for p in range(NPANEL):
                    for h in range(2):
                        raw = st["raws"][(h, p)]
                        r = h * NPANEL + p
                        rl = rlp.tile([1, W], f32, tag="rl", name=f"rl{r}")
                        nc.gpsimd.dma_start(rl[:], recipl[r:r + 1, :])
                        bc = bcp.tile([64, W], f32, tag="bc")
                        nc.gpsimd.partition_broadcast(bc[:], rl[0:1, :], channels=64)
                        if h == 0:
                            nc.vector.tensor_mul(attnT[0:64, p * W:(p + 1) * W], raw[0:64, :], bc[:])
                        else:
                            stg = stgp.tile([64, W], b16, tag="stg")
                            nc.vector.tensor_mul(stg[:], raw[0:64, :], bc[:])
                            nc.gpsimd.dma_start(attnT[64:128, p * W:(p + 1) * W], stg[:])
                st["attnT"] = attnT

            def emit_proj(b):
                attnT = state[b]["attnT"]
                for j in range(NTOK):
                    osb = osbp.tile([128, D], f32, tag="osb")
                    for n in range(D // W):
                        ps = mmp.tile([128, W], f32, tag="mm")
                        nc.tensor.matmul(
                            ps[:], attnT[:, j * 128:(j + 1) * 128],
                            wp_sb[:, n * W:(n + 1) * W], start=True, stop=True)
                        nc.any.tensor_copy(osb[:, n * W:(n + 1) * W], ps[:])
                    nc.scalar.dma_start(out_d[b, j * 128:(j + 1) * 128, :], osb[:])
                del state[b]

            # batch-level software pipeline: qkv(b+1) is emitted before
            # proj(b) so the PE never head-of-line blocks on the recip tail
            emit_qkv(0)
            for b in range(Bv):
                emit_attention(b)
                emit_scale(b)
                if b + 1 < Bv:
                    emit_qkv(b + 1)
                emit_proj(b)

    nc.compile()
    return nc


def prep_core_inputs(x, attn_mask, w_qkv, w_proj):
    """Host-side shard prep. Returns list of 8 in_maps."""
    Bv, Tv, Dv = x.shape
    xt = np.ascontiguousarray(x.transpose(0, 2, 1)).astype(bf16)
    ql = np.arange(W)
    kl = np.arange(W)
    m1 = (ql[None, :] >= kl[:, None]).astype(bf16)
    maskt = np.concatenate([m1, m1], axis=1)  # duplicated for the 2 packed heads
    in_maps = []
    for c in range(NCORES):
        in_maps.append({
            "xt": xt,
            "wq": np.ascontiguousarray(w_qkv[:, CD * c:CD * (c + 1)]).astype(bf16),
            "wk": np.ascontiguousarray(w_qkv[:, Dv + CD * c:Dv + CD * (c + 1)]).astype(bf16),
            "wv": np.ascontiguousarray(w_qkv[:, 2 * Dv + CD * c:2 * Dv + CD * (c + 1)]).astype(bf16),
            "wp": np.ascontiguousarray(w_proj[CD * c:CD * (c + 1), :]).astype(bf16),
            "maskt": np.ascontiguousarray(maskt),
        })
    return in_maps


def check_causal(attn_mask):
    m = np.asarray(attn_mask)[0, 0]
    Tv = m.shape[0]
    tril = np.tril(np.ones((Tv, Tv), bool))
    return bool(np.all(m[tril] == 0.0)) and bool(np.all(m[~tril] <= np.float32(-1e30)))


def kernel(x, attn_mask, w_qkv, w_proj):
    import os

    from concourse.bass_utils import run_bass_kernel_spmd

    global LAST_RESULT
    x = np.asarray(x)
    attn_mask = np.asarray(attn_mask)
    w_qkv = np.asarray(w_qkv)
    w_proj = np.asarray(w_proj)
    if not check_causal(attn_mask):
        raise NotImplementedError("kernel compiled for causal attn_mask")

    key = (x.shape[0], x.shape[1])
    if key not in _PROG_CACHE:
        _PROG_CACHE[key] = build_program(Bv=x.shape[0], Tv=x.shape[1])
    nc = _PROG_CACHE[key]

    in_maps = prep_core_inputs(x, attn_mask, w_qkv, w_proj)
    kwargs = {}
    if os.environ.get("MHSA_TRACE"):
        _install_ntff_hook()
        kwargs = {"trace": True, "tmpdir": os.environ.get("MHSA_TRACE_DIR") or None}
    res = run_bass_kernel_spmd(nc, in_maps, list(range(NCORES)), **kwargs)
    LAST_RESULT = res
    out = np.zeros((x.shape[0], x.shape[1], D), np.float32)
    for c in range(NCORES):
        out += res.results[c]["out"]
    return out
